# revision 42
# baseline (speedup 1.0000x reference)
"""AttentionBlock kernel for 8 TRN2 NeuronCores — t-split + fp8 DoubleRow + folded QK.

Reference (per batch b, T=2048, D=HID=1024):
    x = minibatch[b].T                      # [T, HID]
    m = x @ emb_w.T + emb_b                 # [T, D]
    K/Q/V = m @ W.T + b  (emb folded into combined weights on the host)
    logits = Q @ K.T  masked to t >= s else -32767
    probs = softmax(logits, axis=t) / 32    # softmax over the QUERY axis t
    read = probs @ V                        # contract over s
    out[b] = (read + m).T                   # [D, T]

Distribution: core c = 2*b + h owns batch b and the t-blocks {128*(2u+h)}
(interleaved for causal balance).  The softmax normalization (over t!) needs
cross-core stats; one tiny AllGather exchanges (-M, Z) per s-block and
f[s] = exp(M_loc - M_glob) / (32 * Z_glob) is folded into V.

Folded-QK trick: logits = (x Weq + beq)(x Wek + bek)^T.  Per-s additive
terms cancel in the softmax over t, so
    logits ~ x^T A x + delta[t],   A = Weq Wek^T,  delta = (Weq bek) . x
A is precomputed on the host; G = A^T x is computed on-chip (same cost as
the old Q phase) and the whole K phase disappears.  delta is baked into the
mask tiles (which are added to the logits psum via an fp8e5 DoubleRow
identity matmul).

Precision: fp8 DoubleRow everywhere; m uses a three-term split-fp8 product
(xh@Wh + xl@Wh + xh@Wl at a 16x weight scale).  A is scaled 64x.  E and f*V
are stored e5m2.

All per-core differences (t/s column permutation [own|peer], mask+delta
contents, stat-merge blend weights) enter via input DATA — the graph is
SPMD-identical.
"""

import sys

for _p in ("/opt/trn_rl_repo", "/opt/pypackages"):
    if _p not in sys.path:
        sys.path.insert(0, _p)

import numpy as np
import ml_dtypes

import concourse.bass as bass
import concourse.mybir as mybir
import concourse.tile as tile
from concourse import bacc
from concourse.bass_utils import run_bass_kernel_spmd

B, HID, T, D = 4, 1024, 2048, 1024
P = 128
SB = 16          # s-blocks of 128 (full T) per core
OT = 1024        # own t columns per core
NEGM = -57344.0  # additive mask value (exact in e5m2; acts as -inf through exp)
WS = 16.0        # host-side fp8 weight scale (wv / mw)
SA = 64.0        # host-side fp8 scale for A

BF = mybir.dt.bfloat16
F8 = mybir.dt.float8e4
E5 = mybir.dt.float8e5
F32 = mybir.dt.float32
DR = mybir.MatmulPerfMode.DoubleRow

PROFILE = False
LAST_EXEC_NS = None
_CACHE = {}


def _build_nc():
    nc = bacc.Bacc(None, target_bir_lowering=False, debug=False)

    x8o = nc.declare_dram_parameter("x8o", [512, 2 * OT], F8, isOutput=False)
    x8p_ = nc.declare_dram_parameter("x8p", [512, 2 * OT], F8, isOutput=False)
    xl8 = nc.declare_dram_parameter("xl8", [512, 2 * OT], F8, isOutput=False)
    wa8 = nc.declare_dram_parameter("wa8", [512, 2 * D], F8, isOutput=False)
    wv8 = nc.declare_dram_parameter("wv8", [512, 2 * D], F8, isOutput=False)
    mwh8 = nc.declare_dram_parameter("mwh8", [512, 2 * D], F8, isOutput=False)
    mwl8 = nc.declare_dram_parameter("mwl8", [512, 2 * D], F8, isOutput=False)
    masks = nc.declare_dram_parameter("masks", [(SB + 2) * P, 512], E5, isOutput=False)
    ident8 = nc.declare_dram_parameter("ident8", [P, 2 * P], E5, isOutput=False)
    ones8 = nc.declare_dram_parameter("ones8", [1, 2 * P], F8, isOutput=False)
    vb8 = nc.declare_dram_parameter("vb8", [1, 2 * D], F8, isOutput=False)
    bias4 = nc.declare_dram_parameter("bias4", [P, 10], F32, isOutput=False)
    out_ext = nc.declare_dram_parameter("out", [D, OT], BF, isOutput=True)

    stats_in = nc.dram_tensor("stats_in", [P, 32], F32)
    stats_out = nc.dram_tensor("stats_out", [2 * P, 32], F32)

    Ident = mybir.ActivationFunctionType.Identity
    Exp = mybir.ActivationFunctionType.Exp
    X = mybir.AxisListType.X
    MUL = mybir.AluOpType.mult
    ADD = mybir.AluOpType.add
    MIN = mybir.AluOpType.min
    RG = [[0, 1], [2, 3], [4, 5], [6, 7]]

    with tile.TileContext(nc) as tc:
        with (
            tc.tile_pool(name="const", bufs=1) as const,
            tc.tile_pool(name="xbig", bufs=1) as xbig,
            tc.tile_pool(name="wbig", bufs=3) as wbig,
            tc.tile_pool(name="gp", bufs=1) as gp,
            tc.tile_pool(name="ep", bufs=8) as ep,
            tc.tile_pool(name="vp", bufs=8) as vp,
            tc.tile_pool(name="vcp", bufs=8) as vcp,
            tc.tile_pool(name="mp", bufs=8) as mp,
            tc.tile_pool(name="sxp", bufs=1) as sxp,
            tc.tile_pool(name="osp", bufs=6) as osp,
            tc.tile_pool(name="ps1", bufs=3, space="PSUM") as ps1,
            tc.tile_pool(name="ps5", bufs=2, space="PSUM") as ps5,
        ):
            # ---- startup: x8 own-half + A weights as j-pair DMAs ----
            xot = xbig.tile([P, 4, 2, OT], F8, tag="xot")
            wat = xbig.tile([P, 4, 2, D], F8, tag="wat")
            xsrc = x8o.rearrange("(j p) (a t) -> p j a t", j=4, a=2)
            wsrc = wa8.rearrange("(j p) (a d) -> p j a d", j=4, a=2)
            nc.sync.dma_start(xot[:, 0, :, 0:512], xsrc[:, 0, :, 0:512])
            nc.sync.dma_start(wat[:, 0, :, 0:512], wsrc[:, 0, :, 0:512])
            nc.sync.dma_start(xot[:, 0, :, 512:1024], xsrc[:, 0, :, 512:1024])
            nc.sync.dma_start(wat[:, 0, :, 512:1024], wsrc[:, 0, :, 512:1024])
            for j in range(1, 4):
                nc.sync.dma_start(xot[:, j], xsrc[:, j])
                nc.sync.dma_start(wat[:, j], wsrc[:, j])

            # ---- G = A^T x (own t cols), e4m3, epilogues spread over engines ----
            gt = gp.tile([P, 4, 2, OT], F8, tag="g")
            for r in range(8):
                pt = ps1.tile([P, 1024], F32, tag="p1", name=f"psg{r}")
                for i in range(2):
                    for j in range(4):
                        nc.tensor.matmul(
                            pt[:, i * 512 : (i + 1) * 512],
                            wat[:, j, :, r * P : (r + 1) * P],
                            xot[:, j, :, i * 512 : (i + 1) * 512],
                            start=(j == 0), stop=(j == 3), perf_mode=DR,
                        )
                dst = gt[:, r // 2, r % 2, :]
                if r % 2 == 0:
                    nc.scalar.activation(dst, pt[:], Ident, scale=1.0 / SA)
                else:
                    nc.vector.tensor_scalar_mul(dst, pt[:], 1.0 / SA)

            # ---- remaining input DMAs (merged; consumed in later phases) ----
            # masks first: the first logits block depends on them
            mskt = const.tile([P, SB + 2, 512], E5)
            nc.sync.dma_start(
                mskt[:], masks.rearrange("(l p) c -> p l c", l=SB + 2)
            )
            identt = const.tile([P, 2, P], E5)
            nc.sync.dma_start(identt[:], ident8.rearrange("p (a q) -> p a q", a=2))
            b4 = const.tile([P, 10], F32)
            nc.sync.dma_start(b4[:], bias4[:])
            ebt, wtbt = b4[:, 0:8], b4[:, 8:10]
            xpt = xbig.tile([P, 4, 2, OT], F8, tag="xp")
            nc.sync.dma_start(
                xpt[:], x8p_.rearrange("(j p) (a t) -> p j a t", j=4, a=2)
            )
            wvt = wbig.tile([P, 4, 2, D], F8, tag="wbig", name="wv")
            nc.sync.dma_start(
                wvt[:], wv8.rearrange("(j p) (a d) -> p j a d", j=4, a=2)
            )
            onest = const.tile([1, 2, P], F8)
            nc.sync.dma_start(onest[:], ones8.rearrange("o (a q) -> o a q", a=2))
            vbt = const.tile([1, 2, D], F8)
            nc.sync.dma_start(vbt[:], vb8.rearrange("o (a d) -> o a d", a=2))
            xlt = xbig.tile([P, 4, 2, OT], F8, tag="xl")
            nc.sync.dma_start(
                xlt[:], xl8.rearrange("(j p) (a t) -> p j a t", j=4, a=2)
            )
            mwht = wbig.tile([P, 4, 2, D], F8, tag="wbig", name="mwh")
            nc.sync.dma_start(
                mwht[:], mwh8.rearrange("(j p) (a d) -> p j a d", j=4, a=2)
            )
            mwlt = wbig.tile([P, 4, 2, D], F8, tag="wbig", name="mwl")
            nc.sync.dma_start(
                mwlt[:], mwl8.rearrange("(j p) (a d) -> p j a d", j=4, a=2)
            )

            # ---- logits + E + local stats, V halves interleaved ----
            spack = sxp.tile([P, 32], F32)        # [M | Z] per s-block column
            mpack, zpack = spack[:, 0:16], spack[:, 16:32]
            et = [ep.tile([P, 2, OT], E5, tag="e", name=f"e{j}") for j in range(8)]
            mt = [mp.tile([P, OT], BF, tag="m", name=f"m{d}") for d in range(8)]
            vt = [vp.tile([P, 2, D], BF, tag="v", name=f"v{j}") for j in range(8)]

            def xs_lhs(l, j):
                # stationary [P, 2, 128] slice of x for s-block l
                if l < 8:
                    return xot[:, j, :, (l % 8) * P : ((l % 8) + 1) * P]
                return xpt[:, j, :, (l % 8) * P : ((l % 8) + 1) * P]

            def v_half(l, i, eng):
                # alternate psum pools so evacuation never throttles the PE
                if i == 0:
                    pt = ps5.tile([P, 512], F32, tag="p5", name=f"psv{l}_{i}")
                else:
                    pt = ps1.tile([P, 1024], F32, tag="p1", name=f"psv{l}_{i}")
                    pt = pt[:, 0:512]
                for j in range(4):
                    nc.tensor.matmul(
                        pt[:], xs_lhs(l, j),
                        wvt[:, j, :, i * 512 : (i + 1) * 512],
                        start=(j == 0), stop=False, perf_mode=DR,
                    )
                nc.tensor.matmul(
                    pt[:], onest[0:1], vbt[0:1, :, i * 512 : (i + 1) * 512],
                    start=False, stop=True, perf_mode=DR,
                )
                dst = vt[l // 2][:, l % 2, i * 512 : (i + 1) * 512]
                if eng == 0:
                    nc.scalar.activation(dst, pt[:], Ident, scale=1.0 / WS)
                else:
                    nc.vector.tensor_scalar_mul(dst, pt[:], 1.0 / WS)

            def logits_block(l):
                bnd = (l % 8) // 4      # boundary tile index == first computed
                pt = ps1.tile([P, 1024], F32, tag="p1", name=f"psl{l}")
                for i in range(bnd, 2):
                    for j in range(4):
                        nc.tensor.matmul(
                            pt[:, i * 512 : (i + 1) * 512],
                            xs_lhs(l, j),
                            gt[:, j, :, i * 512 : (i + 1) * 512],
                            start=(j == 0), stop=False, perf_mode=DR,
                        )
                    mrow = l if i == bnd else SB    # boundary mask / delta-only
                    # plane 1 of the rhs is multiplied by the zero half of
                    # identt, so any initialized row works — use mrow+1
                    nc.tensor.matmul(
                        pt[:, i * 512 : (i + 1) * 512],
                        identt[:], mskt[:, mrow : mrow + 2, :],
                        start=False, stop=True, perf_mode=DR,
                    )
                sl = pt[:, bnd * 512 : 1024]
                nc.vector.reduce_max(mpack[:, l : l + 1], sl, axis=X)
                negM = sxp.tile([P, 1], F32, tag="ng", bufs=SB, name=f"ng{l}")
                nc.gpsimd.tensor_scalar_mul(negM, mpack[:, l : l + 1], -1.0)
                nc.scalar.activation(
                    et[l // 2][:, l % 2, bnd * 512 : 1024], sl, Exp,
                    bias=negM[:, 0:1], accum_out=zpack[:, l : l + 1],
                )

            # V block schedule: pairs {0,1,4,5} (= s-blocks 0-3, 8-11) first
            for l in range(SB):
                logits_block(l)

            # ---- stats exchange (one tiny AllGather per pair) ----
            nc.sync.dma_start(stats_in[:], spack[:])
            nc.gpsimd.collective_compute(
                "AllGather", mybir.AluOpType.bypass,
                ins=[stats_in[:]], outs=[stats_out[:]], replica_groups=RG,
            )
            gall = sxp.tile([P, 2, 32], F32)
            nc.sync.dma_start(gall[:], stats_out.rearrange("(r p) c -> p r c", r=2))
            gtop, gbot = gall[:, 0, :], gall[:, 1, :]

            # ---- V phase: fills the collective window (PE + Act/DVE epis) ----
            for l in range(SB):
                v_half(l, 0, l % 2)
                v_half(l, 1, 1 - l % 2)

            # ---- combine stats -> f[s] = WS * exp(M_loc - Mg) / (32 Zg) ----
            oth = sxp.tile([P, 32], F32)
            nc.vector.tensor_scalar_mul(oth[:], gtop[:], wtbt[:, 0:1])
            nc.vector.scalar_tensor_tensor(
                oth[:], gbot[:], wtbt[:, 1:2], oth[:], op0=MUL, op1=ADD
            )
            # peer stats, s-halves swapped (peer's block l <-> own block l+-8)
            oM = sxp.tile([P, 16], F32)
            oZ = sxp.tile([P, 16], F32)
            nc.vector.tensor_copy(oM[:, 0:8], oth[:, 8:16])
            nc.vector.tensor_copy(oM[:, 8:16], oth[:, 0:8])
            nc.vector.tensor_copy(oZ[:, 0:8], oth[:, 24:32])
            nc.vector.tensor_copy(oZ[:, 8:16], oth[:, 16:24])
            mg = sxp.tile([P, 16], F32)
            nc.vector.tensor_max(mg[:], mpack[:], oM[:])
            dd = sxp.tile([P, 32], F32)
            nc.vector.tensor_sub(dd[:, 0:16], mpack[:], mg[:])   # Ml - Mg
            nc.vector.tensor_sub(dd[:, 16:32], oM[:], mg[:])     # Mo - Mg
            ee = sxp.tile([P, 32], F32)
            nc.scalar.activation(ee[:], dd[:], Exp)              # [expm | expo]
            zz = sxp.tile([P, 32], F32)
            nc.vector.tensor_mul(zz[:, 0:16], zpack[:], ee[:, 0:16])
            nc.vector.tensor_mul(zz[:, 16:32], oZ[:], ee[:, 16:32])
            zg = sxp.tile([P, 16], F32)
            nc.vector.tensor_add(zg[:], zz[:, 0:16], zz[:, 16:32])
            rz = sxp.tile([P, 16], F32)
            nc.vector.reciprocal(rz[:], zg[:])
            fsc = sxp.tile([P, 16], F32)
            nc.vector.scalar_tensor_tensor(
                fsc[:], rz[:], WS / 32.0, ee[:, 0:16], op0=MUL, op1=MUL
            )

            # ---- Vc = (WS*f) * V (e5m2); read psums then hold WS*(read[+m]) ----
            vct = [vcp.tile([P, 2, D], E5, tag="vc", name=f"vc{j}") for j in range(8)]

            def vc_half(j2, hd, eng):
                # [P, 2, 512] d-half of a pair: read tile d-blocks consume
                # cols d*128..; h0-first ordering lets read0 start early
                for a in range(2):
                    l = 2 * j2 + a
                    dst = vct[j2][:, a, hd * 512 : (hd + 1) * 512]
                    src = vt[j2][:, a, hd * 512 : (hd + 1) * 512]
                    if eng == 0:
                        nc.scalar.activation(dst, src, Ident, scale=fsc[:, l : l + 1])
                    elif eng == 1:
                        nc.vector.tensor_scalar_mul(dst, src, fsc[:, l : l + 1])
                    else:
                        nc.gpsimd.tensor_scalar_mul(dst, src, fsc[:, l : l + 1])

            # ordered by when read consumes them; spread over DVE/Act/Pool.
            # Pool (slowest) gets the late pairs {2,3,6,7} so Act/DVE stay
            # free for read-psum evacuation.
            vc_half(0, 0, 1)
            vc_half(1, 0, 0)
            vc_half(4, 0, 1)
            vc_half(5, 0, 0)
            vc_half(0, 1, 1)
            vc_half(1, 1, 0)
            vc_half(4, 1, 1)
            vc_half(5, 1, 0)
            for hd in range(2):
                for j2 in (2, 3):
                    vc_half(j2, hd, 2)
            # pairs {6,7} are emitted inside the read0 loop (DVE/Act slack)

            # ---- m matmuls (split-fp8: xh@Wh + xl@Wh + xh@Wl) ----
            M_TERMS = 3
            def m_mms(pt, d, i, stop):
                nmm = 0
                for j in range(4):
                    for lhs, rhs in (
                        (mwht, xot[:, j, :, i * 512 : (i + 1) * 512]),
                        (mwlt, xot[:, j, :, i * 512 : (i + 1) * 512]),
                        (mwht, xlt[:, j, :, i * 512 : (i + 1) * 512]),
                    )[:M_TERMS]:
                        nmm += 1
                        nc.tensor.matmul(
                            pt[:], lhs[:, j, :, d * P : (d + 1) * P], rhs,
                            start=(nmm == 1), stop=(stop and nmm == 4 * M_TERMS),
                            perf_mode=DR,
                        )

            # even-d m blocks stay a separate phase (fills the collective
            # window); odd-d m matmuls are folded into the read psums, whose
            # epilogue then runs on Act (bias+scale) instead of a DVE stt.
            # Pre-issue the read0 odd-d m-parts now: vct-independent PE work.
            pre0 = {}
            for d in (1, 3, 5):
                pt = ps1.tile([P, 1024], F32, tag="p1", name=f"psr0_{d}")
                pre0[d] = pt[:, 0:512]
                m_mms(pre0[d], d, 0, stop=False)

            for d in (0, 2, 4, 6):
                for i in range(2):
                    pt = ps5.tile([P, 512], F32, tag="p5", name=f"psm{d}_{i}")
                    m_mms(pt, d, i, stop=True)
                    nc.scalar.activation(
                        mt[d][:, i * 512 : (i + 1) * 512], pt[:], Ident,
                        bias=ebt[:, d : d + 1], scale=1.0 / WS,
                    )

            # ---- read: psum[d, t] = WS*(read + m);  out = read + m ----
            def read_psum(i, d, pt, prs, started):
                for idx, j2 in enumerate(prs):
                    nc.tensor.matmul(
                        pt[:],
                        vct[j2][:, :, d * P : (d + 1) * P],
                        et[j2][:, :, i * 512 : (i + 1) * 512],
                        start=(idx == 0 and not started),
                        stop=(idx == len(prs) - 1),
                        perf_mode=DR,
                    )

            def read_evac(i, d, pt, folded):
                osb = osp.tile([P, 512], BF, tag="os", name=f"os{i}_{d}")
                if folded:
                    nc.scalar.activation(
                        osb[:], pt[:], Ident,
                        bias=ebt[:, d : d + 1], scale=1.0 / WS,
                    )
                else:
                    nc.vector.scalar_tensor_tensor(
                        osb[:], pt[:], 1.0 / WS,
                        mt[d][:, i * 512 : (i + 1) * 512], op0=MUL, op1=ADD,
                    )
                nc.sync.dma_start(
                    out_ext[d * P : (d + 1) * P, i * 512 : (i + 1) * 512], osb[:]
                )

            PRS0 = [0, 1, 4, 5]
            PRS1 = [0, 1, 4, 5, 2, 3, 6, 7]
            # vc pairs {6,7} (needed by read1) fill DVE/Act slack between
            # read0 evacuations
            vc67 = {0: (6, 0, 1), 1: (7, 0, 0), 2: (6, 1, 1), 3: (7, 1, 0)}
            # read tile 0: odd d pre-folded above
            for d in range(8):
                if d % 2 == 1 and d != 7:
                    pt = pre0[d]
                    read_psum(0, d, pt, PRS0, started=True)
                    read_evac(0, d, pt, folded=True)
                else:
                    pt = ps5.tile([P, 512], F32, tag="p5", name=f"psr0_{d}")
                    if d == 7:
                        m_mms(pt, d, 0, stop=False)
                        read_psum(0, d, pt, PRS0, started=True)
                        read_evac(0, d, pt, folded=True)
                    else:
                        read_psum(0, d, pt, PRS0, started=False)
                        read_evac(0, d, pt, folded=False)
                if d in vc67:
                    vc_half(*vc67[d])
            # read tile 1: folded odd-d psums first (their m-parts keep the
            # PE busy while the late vc pairs drain); last two groups end on
            # different engines (Act for d7, DVE for d6) to shorten the tail
            for d in (1, 3, 5):
                pt = ps5.tile([P, 512], F32, tag="p5", name=f"psr1_{d}")
                m_mms(pt, d, 1, stop=False)
                read_psum(1, d, pt, PRS1, started=True)
                read_evac(1, d, pt, folded=True)
            for d in (0, 2, 4):
                pt = ps1.tile([P, 1024], F32, tag="p1", name=f"psr1_{d}")
                pt = pt[:, 0:512]
                read_psum(1, d, pt, PRS1, started=False)
                read_evac(1, d, pt, folded=False)
            pt = ps5.tile([P, 512], F32, tag="p5", name="psr1_7")
            m_mms(pt, 7, 1, stop=False)
            read_psum(1, 7, pt, PRS1, started=True)
            read_evac(1, 7, pt, folded=True)
            pt = ps1.tile([P, 1024], F32, tag="p1", name="psr1_6")
            pt = pt[:, 0:512]
            read_psum(1, 6, pt, PRS1, started=False)
            read_evac(1, 6, pt, folded=False)

    nc.compile()
    return nc


def _prep_inputs(minibatch, emb_w, emb_b, key_w, key_b, query_w, query_b,
                 value_w, value_b):
    bf = ml_dtypes.bfloat16
    f8 = ml_dtypes.float8_e4m3
    e5 = ml_dtypes.float8_e5m2
    ewT = np.ascontiguousarray(emb_w.T).astype(np.float64)
    W_eq = ewT @ query_w.T.astype(np.float64)
    W_ek = ewT @ key_w.T.astype(np.float64)
    W_ev = (ewT @ value_w.T.astype(np.float64)).astype(np.float32)
    b_ek = emb_b.astype(np.float64) @ key_w.T.astype(np.float64) + key_b.astype(np.float64)
    b_ev = (emb_b @ value_w.T + value_b).astype(np.float32)

    A = W_eq @ W_ek.T               # [HID, HID]; exact in f64
    u = W_eq @ b_ek                 # [HID]; delta[t] = u . x[:, t]

    def pack_w(W):
        # [HID, D] -> [512, 2D]: row 128j+p, col a*D+d  holds W[256j+128a+p, d]
        W4 = np.asarray(W, dtype=np.float32).reshape(4, 2, P, -1)
        return np.ascontiguousarray(
            W4.transpose(0, 2, 1, 3).reshape(512, -1)
        ).astype(f8)

    def pack_bias(v):
        return np.ascontiguousarray(v.reshape(8, P).T).astype(np.float32)

    # 16x scale lifts the ~0.013-rms folded weights out of fp8-subnormal range
    ewT32 = ewT.astype(np.float32)
    mwh_f32 = (WS * ewT32).astype(f8).astype(np.float32)
    shared = {
        "wa8": pack_w(SA * A),
        "wv8": pack_w(WS * W_ev),
        "mwh8": pack_w(mwh_f32),
        "mwl8": pack_w(WS * ewT32 - mwh_f32),
        "vb8": np.concatenate([WS * b_ev, np.zeros(D, np.float32)])[None, :].astype(f8),
        "ones8": np.concatenate(
            [np.ones(P, np.float32), np.zeros(P, np.float32)])[None, :].astype(f8),
        "ident8": np.concatenate(
            [np.eye(P, dtype=np.float32), np.zeros((P, P), np.float32)], axis=1
        ).astype(e5),
    }

    in_maps = []
    for c in range(8):
        b, h = c // 2, c % 2
        xbT = minibatch[b].astype(np.float32)          # [HID, T]
        own = np.concatenate(
            [np.arange(P * (2 * u_ + h), P * (2 * u_ + h) + P) for u_ in range(8)]
        )
        peer = np.concatenate(
            [np.arange(P * (2 * u_ + 1 - h), P * (2 * u_ + 1 - h) + P) for u_ in range(8)]
        )
        xo = xbT[:, own]                               # [HID, 1024] own-t cols
        xpe = xbT[:, peer]

        def pack_x(xc):
            x4 = xc.reshape(4, 2, P, xc.shape[1])
            return np.ascontiguousarray(
                x4.transpose(0, 2, 1, 3).reshape(512, -1)
            ).astype(f8)

        x8o_c = pack_x(xo)
        x8p_c = pack_x(xpe)
        xh = xo.astype(f8).astype(np.float32)
        xl8_c = pack_x(xo - xh)                        # split-fp8 residual of x

        delta = (u @ xo.astype(np.float64)).astype(np.float32)   # [1024]

        # masks: per s-block l (permuted order), boundary-tile content + delta;
        # row SB = delta-only tile for the non-boundary region of bnd=0 blocks;
        # row SB+1 = dummy (matmul'd against the zero half of the identity).
        mk = np.zeros(((SB + 2) * P, 512), dtype=np.float32)
        for l in range(SB):
            bnd = (l % 8) // 4
            base_pos = 4 * bnd
            if l < 8:
                phys = 2 * l + h                 # own-parity s block
            else:
                phys = 2 * (l - 8) + (1 - h)     # peer-parity s block
            srow = P * phys + np.arange(P)[:, None]
            blk = np.zeros((P, 512), dtype=np.float32)
            for pos in range(base_pos, base_pos + 4):
                tcol = P * (2 * pos + h) + np.arange(P)[None, :]
                blk[:, (pos - base_pos) * P : (pos - base_pos + 1) * P] = (
                    (tcol < srow) * NEGM
                )
            blk += delta[bnd * 512 : bnd * 512 + 512][None, :]
            mk[l * P : (l + 1) * P, :] = blk
        mk[SB * P : (SB + 1) * P, :] = delta[512:1024][None, :]

        bias4c = np.zeros((P, 10), dtype=np.float32)
        bias4c[:, 0:8] = pack_bias(emb_b.astype(np.float32))
        bias4c[:, 8] = 1.0 if h == 1 else 0.0    # weight of gathered rank0 rows
        bias4c[:, 9] = 1.0 if h == 0 else 0.0    # weight of gathered rank1 rows
        in_maps.append(dict(
            shared,
            x8o=x8o_c,
            x8p=x8p_c,
            xl8=xl8_c,
            masks=mk.astype(e5),
            bias4=bias4c,
        ))
    return in_maps


def kernel(**inputs):
    global LAST_EXEC_NS
    inputs = {k: np.asarray(v) for k, v in inputs.items()}
    if "nc" not in _CACHE:
        _CACHE["nc"] = _build_nc()
    nc = _CACHE["nc"]
    in_maps = _prep_inputs(**inputs)
    kw = {}
    if PROFILE:
        kw["trace"] = True
    res = run_bass_kernel_spmd(nc, in_maps, core_ids=list(range(8)), **kw)
    LAST_EXEC_NS = getattr(res, "exec_time_ns", None)
    out = np.empty((B, D, T), dtype=np.float32)
    for c in range(8):
        b, h = c // 2, c % 2
        o = np.asarray(res.results[c]["out"]).astype(np.float32)  # [D, OT]
        own = np.concatenate(
            [np.arange(P * (2 * u_ + h), P * (2 * u_ + h) + P) for u_ in range(8)]
        )
        out[b][:, own] = o
    return out


# revision 44
# speedup vs baseline: 1.0061x; 1.0061x over previous
"""AttentionBlock kernel for 8 TRN2 NeuronCores — t-split + fp8 DoubleRow + folded QK.

Reference (per batch b, T=2048, D=HID=1024):
    x = minibatch[b].T                      # [T, HID]
    m = x @ emb_w.T + emb_b                 # [T, D]
    K/Q/V = m @ W.T + b  (emb folded into combined weights on the host)
    logits = Q @ K.T  masked to t >= s else -32767
    probs = softmax(logits, axis=t) / 32    # softmax over the QUERY axis t
    read = probs @ V                        # contract over s
    out[b] = (read + m).T                   # [D, T]

Distribution: core c = 2*b + h owns batch b and the t-blocks {128*(2u+h)}
(interleaved for causal balance).  The softmax normalization (over t!) needs
cross-core stats; one tiny AllGather exchanges (-M, Z) per s-block and
f[s] = exp(M_loc - M_glob) / (32 * Z_glob) is folded into V.

Folded-QK trick: logits = (x Weq + beq)(x Wek + bek)^T.  Per-s additive
terms cancel in the softmax over t, so
    logits ~ x^T A x + delta[t],   A = Weq Wek^T,  delta = (Weq bek) . x
A is precomputed on the host; G = A^T x is computed on-chip (same cost as
the old Q phase) and the whole K phase disappears.  delta is baked into the
mask tiles (which are added to the logits psum via an fp8e5 DoubleRow
identity matmul).

Precision: fp8 DoubleRow everywhere; m uses a three-term split-fp8 product
(xh@Wh + xl@Wh + xh@Wl at a 16x weight scale).  A is scaled 64x.  E and f*V
are stored e5m2.

All per-core differences (t/s column permutation [own|peer], mask+delta
contents, stat-merge blend weights) enter via input DATA — the graph is
SPMD-identical.
"""

import sys

for _p in ("/opt/trn_rl_repo", "/opt/pypackages"):
    if _p not in sys.path:
        sys.path.insert(0, _p)

import numpy as np
import ml_dtypes

import concourse.bass as bass
import concourse.mybir as mybir
import concourse.tile as tile
from concourse import bacc
from concourse.bass_utils import run_bass_kernel_spmd

B, HID, T, D = 4, 1024, 2048, 1024
P = 128
SB = 16          # s-blocks of 128 (full T) per core
OT = 1024        # own t columns per core
NEGM = -57344.0  # additive mask value (exact in e5m2; acts as -inf through exp)
WS = 16.0        # host-side fp8 weight scale (wv / mw)
SA = 64.0        # host-side fp8 scale for A

BF = mybir.dt.bfloat16
F8 = mybir.dt.float8e4
E5 = mybir.dt.float8e5
F32 = mybir.dt.float32
DR = mybir.MatmulPerfMode.DoubleRow

PROFILE = False
LAST_EXEC_NS = None
_CACHE = {}


def _build_nc():
    nc = bacc.Bacc(None, target_bir_lowering=False, debug=False)

    x8o = nc.declare_dram_parameter("x8o", [512, 2 * OT], F8, isOutput=False)
    x8p_ = nc.declare_dram_parameter("x8p", [512, 2 * OT], F8, isOutput=False)
    xl8 = nc.declare_dram_parameter("xl8", [512, 2 * OT], F8, isOutput=False)
    wa8 = nc.declare_dram_parameter("wa8", [512, 2 * D], F8, isOutput=False)
    wv8 = nc.declare_dram_parameter("wv8", [512, 2 * D], F8, isOutput=False)
    mwh8 = nc.declare_dram_parameter("mwh8", [512, 2 * D], F8, isOutput=False)
    mwl8 = nc.declare_dram_parameter("mwl8", [512, 2 * D], F8, isOutput=False)
    masks = nc.declare_dram_parameter("masks", [(SB + 2) * P, 512], E5, isOutput=False)
    ident8 = nc.declare_dram_parameter("ident8", [P, 2 * P], E5, isOutput=False)
    ones8 = nc.declare_dram_parameter("ones8", [1, 2 * P], F8, isOutput=False)
    vb8 = nc.declare_dram_parameter("vb8", [1, 2 * D], F8, isOutput=False)
    bias4 = nc.declare_dram_parameter("bias4", [P, 10], F32, isOutput=False)
    out_ext = nc.declare_dram_parameter("out", [D, OT], BF, isOutput=True)

    stats_in = nc.dram_tensor("stats_in", [P, 32], F32)
    stats_out = nc.dram_tensor("stats_out", [2 * P, 32], F32)

    Ident = mybir.ActivationFunctionType.Identity
    Exp = mybir.ActivationFunctionType.Exp
    X = mybir.AxisListType.X
    MUL = mybir.AluOpType.mult
    ADD = mybir.AluOpType.add
    MIN = mybir.AluOpType.min
    RG = [[0, 1], [2, 3], [4, 5], [6, 7]]

    with tile.TileContext(nc) as tc:
        with (
            tc.tile_pool(name="const", bufs=1) as const,
            tc.tile_pool(name="xbig", bufs=1) as xbig,
            tc.tile_pool(name="wbig", bufs=3) as wbig,
            tc.tile_pool(name="gp", bufs=1) as gp,
            tc.tile_pool(name="ep", bufs=8) as ep,
            tc.tile_pool(name="vp", bufs=8) as vp,
            tc.tile_pool(name="vcp", bufs=8) as vcp,
            tc.tile_pool(name="mp", bufs=8) as mp,
            tc.tile_pool(name="sxp", bufs=1) as sxp,
            tc.tile_pool(name="osp", bufs=6) as osp,
            tc.tile_pool(name="ps1", bufs=3, space="PSUM") as ps1,
            tc.tile_pool(name="ps5", bufs=2, space="PSUM") as ps5,
        ):
            # ---- startup: x8 own-half + A weights as j-pair DMAs ----
            xot = xbig.tile([P, 4, 2, OT], F8, tag="xot")
            wat = xbig.tile([P, 4, 2, D], F8, tag="wat")
            xsrc = x8o.rearrange("(j p) (a t) -> p j a t", j=4, a=2)
            wsrc = wa8.rearrange("(j p) (a d) -> p j a d", j=4, a=2)
            nc.sync.dma_start(xot[:, 0, :, 0:512], xsrc[:, 0, :, 0:512])
            nc.sync.dma_start(wat[:, 0, :, 0:512], wsrc[:, 0, :, 0:512])
            nc.sync.dma_start(xot[:, 0, :, 512:1024], xsrc[:, 0, :, 512:1024])
            nc.sync.dma_start(wat[:, 0, :, 512:1024], wsrc[:, 0, :, 512:1024])
            for j in range(1, 4):
                nc.sync.dma_start(xot[:, j], xsrc[:, j])
                nc.sync.dma_start(wat[:, j], wsrc[:, j])

            # ---- G = A^T x (own t cols), e4m3, epilogues spread over engines ----
            gt = gp.tile([P, 4, 2, OT], F8, tag="g")
            for r in range(8):
                pt = ps1.tile([P, 1024], F32, tag="p1", name=f"psg{r}")
                for i in range(2):
                    for j in range(4):
                        nc.tensor.matmul(
                            pt[:, i * 512 : (i + 1) * 512],
                            wat[:, j, :, r * P : (r + 1) * P],
                            xot[:, j, :, i * 512 : (i + 1) * 512],
                            start=(j == 0), stop=(j == 3), perf_mode=DR,
                        )
                dst = gt[:, r // 2, r % 2, :]
                if r % 2 == 0:
                    nc.scalar.activation(dst, pt[:], Ident, scale=1.0 / SA)
                else:
                    nc.vector.tensor_scalar_mul(dst, pt[:], 1.0 / SA)

            # ---- remaining input DMAs (merged; consumed in later phases) ----
            # masks first: the first logits block depends on them
            mskt = const.tile([P, SB + 2, 512], E5)
            nc.sync.dma_start(
                mskt[:], masks.rearrange("(l p) c -> p l c", l=SB + 2)
            )
            identt = const.tile([P, 2, P], E5)
            nc.sync.dma_start(identt[:], ident8.rearrange("p (a q) -> p a q", a=2))
            b4 = const.tile([P, 10], F32)
            nc.sync.dma_start(b4[:], bias4[:])
            ebt, wtbt = b4[:, 0:8], b4[:, 8:10]
            xpt = xbig.tile([P, 4, 2, OT], F8, tag="xp")
            nc.sync.dma_start(
                xpt[:], x8p_.rearrange("(j p) (a t) -> p j a t", j=4, a=2)
            )
            wvt = wbig.tile([P, 4, 2, D], F8, tag="wbig", name="wv")
            nc.sync.dma_start(
                wvt[:], wv8.rearrange("(j p) (a d) -> p j a d", j=4, a=2)
            )
            onest = const.tile([1, 2, P], F8)
            nc.sync.dma_start(onest[:], ones8.rearrange("o (a q) -> o a q", a=2))
            vbt = const.tile([1, 2, D], F8)
            nc.sync.dma_start(vbt[:], vb8.rearrange("o (a d) -> o a d", a=2))
            xlt = xbig.tile([P, 4, 2, OT], F8, tag="xl")
            nc.sync.dma_start(
                xlt[:], xl8.rearrange("(j p) (a t) -> p j a t", j=4, a=2)
            )
            mwht = wbig.tile([P, 4, 2, D], F8, tag="wbig", name="mwh")
            nc.sync.dma_start(
                mwht[:], mwh8.rearrange("(j p) (a d) -> p j a d", j=4, a=2)
            )
            mwlt = wbig.tile([P, 4, 2, D], F8, tag="wbig", name="mwl")
            nc.sync.dma_start(
                mwlt[:], mwl8.rearrange("(j p) (a d) -> p j a d", j=4, a=2)
            )

            # ---- logits + E + local stats, V halves interleaved ----
            spack = sxp.tile([P, 32], F32)        # [M | Z] per s-block column
            mpack, zpack = spack[:, 0:16], spack[:, 16:32]
            et = [ep.tile([P, 2, OT], E5, tag="e", name=f"e{j}") for j in range(8)]
            mt = [mp.tile([P, OT], BF, tag="m", name=f"m{d}") for d in range(8)]
            vt = [vp.tile([P, 2, D], BF, tag="v", name=f"v{j}") for j in range(8)]

            def xs_lhs(l, j):
                # stationary [P, 2, 128] slice of x for s-block l
                if l < 8:
                    return xot[:, j, :, (l % 8) * P : ((l % 8) + 1) * P]
                return xpt[:, j, :, (l % 8) * P : ((l % 8) + 1) * P]

            def v_half(l, i, eng):
                # alternate psum pools so evacuation never throttles the PE
                if i == 0:
                    pt = ps5.tile([P, 512], F32, tag="p5", name=f"psv{l}_{i}")
                else:
                    pt = ps1.tile([P, 1024], F32, tag="p1", name=f"psv{l}_{i}")
                    pt = pt[:, 0:512]
                for j in range(4):
                    nc.tensor.matmul(
                        pt[:], xs_lhs(l, j),
                        wvt[:, j, :, i * 512 : (i + 1) * 512],
                        start=(j == 0), stop=False, perf_mode=DR,
                    )
                nc.tensor.matmul(
                    pt[:], onest[0:1], vbt[0:1, :, i * 512 : (i + 1) * 512],
                    start=False, stop=True, perf_mode=DR,
                )
                dst = vt[l // 2][:, l % 2, i * 512 : (i + 1) * 512]
                if eng == 0:
                    nc.scalar.activation(dst, pt[:], Ident, scale=1.0 / WS)
                else:
                    nc.vector.tensor_scalar_mul(dst, pt[:], 1.0 / WS)

            def logits_block(l):
                bnd = (l % 8) // 4      # boundary tile index == first computed
                pt = ps1.tile([P, 1024], F32, tag="p1", name=f"psl{l}")
                for i in range(bnd, 2):
                    for j in range(4):
                        nc.tensor.matmul(
                            pt[:, i * 512 : (i + 1) * 512],
                            xs_lhs(l, j),
                            gt[:, j, :, i * 512 : (i + 1) * 512],
                            start=(j == 0), stop=False, perf_mode=DR,
                        )
                    mrow = l if i == bnd else SB    # boundary mask / delta-only
                    # plane 1 of the rhs is multiplied by the zero half of
                    # identt, so any initialized row works — use mrow+1
                    nc.tensor.matmul(
                        pt[:, i * 512 : (i + 1) * 512],
                        identt[:], mskt[:, mrow : mrow + 2, :],
                        start=False, stop=True, perf_mode=DR,
                    )
                sl = pt[:, bnd * 512 : 1024]
                nc.vector.reduce_max(mpack[:, l : l + 1], sl, axis=X)
                negM = sxp.tile([P, 1], F32, tag="ng", bufs=SB, name=f"ng{l}")
                nc.gpsimd.tensor_scalar_mul(negM, mpack[:, l : l + 1], -1.0)
                nc.scalar.activation(
                    et[l // 2][:, l % 2, bnd * 512 : 1024], sl, Exp,
                    bias=negM[:, 0:1], accum_out=zpack[:, l : l + 1],
                )

            # V block schedule: pairs {0,1,4,5} (= s-blocks 0-3, 8-11) first
            for l in range(SB):
                logits_block(l)

            # ---- stats exchange (one tiny AllGather per pair) ----
            nc.sync.dma_start(stats_in[:], spack[:])
            nc.gpsimd.collective_compute(
                "AllGather", mybir.AluOpType.bypass,
                ins=[stats_in[:]], outs=[stats_out[:]], replica_groups=RG,
            )
            gall = sxp.tile([P, 2, 32], F32)
            nc.sync.dma_start(gall[:], stats_out.rearrange("(r p) c -> p r c", r=2))
            gtop, gbot = gall[:, 0, :], gall[:, 1, :]

            # ---- V phase: fills the collective window (PE + Act/DVE epis) ----
            for l in range(SB):
                v_half(l, 0, l % 2)
                v_half(l, 1, 1 - l % 2)

            # ---- combine stats -> f[s] = WS * exp(M_loc - Mg) / (32 Zg) ----
            oth = sxp.tile([P, 32], F32)
            nc.vector.tensor_scalar_mul(oth[:], gtop[:], wtbt[:, 0:1])
            nc.vector.scalar_tensor_tensor(
                oth[:], gbot[:], wtbt[:, 1:2], oth[:], op0=MUL, op1=ADD
            )
            # peer stats, s-halves swapped (peer's block l <-> own block l+-8)
            oM = sxp.tile([P, 16], F32)
            oZ = sxp.tile([P, 16], F32)
            nc.vector.tensor_copy(oM[:, 0:8], oth[:, 8:16])
            nc.vector.tensor_copy(oM[:, 8:16], oth[:, 0:8])
            nc.vector.tensor_copy(oZ[:, 0:8], oth[:, 24:32])
            nc.vector.tensor_copy(oZ[:, 8:16], oth[:, 16:24])
            mg = sxp.tile([P, 16], F32)
            nc.vector.tensor_max(mg[:], mpack[:], oM[:])
            dd = sxp.tile([P, 32], F32)
            nc.vector.tensor_sub(dd[:, 0:16], mpack[:], mg[:])   # Ml - Mg
            nc.vector.tensor_sub(dd[:, 16:32], oM[:], mg[:])     # Mo - Mg
            ee = sxp.tile([P, 32], F32)
            nc.scalar.activation(ee[:], dd[:], Exp)              # [expm | expo]
            zz = sxp.tile([P, 32], F32)
            nc.vector.tensor_mul(zz[:, 0:16], zpack[:], ee[:, 0:16])
            nc.vector.tensor_mul(zz[:, 16:32], oZ[:], ee[:, 16:32])
            zg = sxp.tile([P, 16], F32)
            nc.vector.tensor_add(zg[:], zz[:, 0:16], zz[:, 16:32])
            rz = sxp.tile([P, 16], F32)
            nc.vector.reciprocal(rz[:], zg[:])
            fsc = sxp.tile([P, 16], F32)
            nc.vector.scalar_tensor_tensor(
                fsc[:], rz[:], WS / 32.0, ee[:, 0:16], op0=MUL, op1=MUL
            )

            # ---- Vc = (WS*f) * V (e5m2); read psums then hold WS*(read[+m]) ----
            vct = [vcp.tile([P, 2, D], E5, tag="vc", name=f"vc{j}") for j in range(8)]

            def vc_half(j2, hd, eng):
                # [P, 2, 512] d-half of a pair: read tile d-blocks consume
                # cols d*128..; h0-first ordering lets read0 start early
                for a in range(2):
                    l = 2 * j2 + a
                    dst = vct[j2][:, a, hd * 512 : (hd + 1) * 512]
                    src = vt[j2][:, a, hd * 512 : (hd + 1) * 512]
                    if eng == 0:
                        nc.scalar.activation(dst, src, Ident, scale=fsc[:, l : l + 1])
                    elif eng == 1:
                        nc.vector.tensor_scalar_mul(dst, src, fsc[:, l : l + 1])
                    else:
                        nc.gpsimd.tensor_scalar_mul(dst, src, fsc[:, l : l + 1])

            # ordered by when read consumes them; spread over DVE/Act/Pool.
            # Pool (slowest) gets the late pairs {2,3,6,7} so Act/DVE stay
            # free for read-psum evacuation.
            vc_half(0, 0, 1)
            vc_half(1, 0, 0)
            vc_half(4, 0, 1)
            vc_half(5, 0, 0)
            vc_half(0, 1, 1)
            vc_half(1, 1, 0)
            vc_half(4, 1, 1)
            vc_half(5, 1, 0)
            for hd in range(2):
                for j2 in (2, 3, 6, 7):
                    vc_half(j2, hd, 2)

            # ---- m matmuls (split-fp8: xh@Wh + xl@Wh + xh@Wl) ----
            M_TERMS = 3
            def m_mms(pt, d, i, stop):
                nmm = 0
                for j in range(4):
                    for lhs, rhs in (
                        (mwht, xot[:, j, :, i * 512 : (i + 1) * 512]),
                        (mwlt, xot[:, j, :, i * 512 : (i + 1) * 512]),
                        (mwht, xlt[:, j, :, i * 512 : (i + 1) * 512]),
                    )[:M_TERMS]:
                        nmm += 1
                        nc.tensor.matmul(
                            pt[:], lhs[:, j, :, d * P : (d + 1) * P], rhs,
                            start=(nmm == 1), stop=(stop and nmm == 4 * M_TERMS),
                            perf_mode=DR,
                        )

            # even-d m blocks stay a separate phase (fills the collective
            # window); odd-d m matmuls are folded into the read psums, whose
            # epilogue then runs on Act (bias+scale) instead of a DVE stt.
            # Pre-issue the read0 odd-d m-parts now: vct-independent PE work.
            pre0 = {}
            for d in (1, 3, 5):
                pt = ps1.tile([P, 1024], F32, tag="p1", name=f"psr0_{d}")
                pre0[d] = pt[:, 0:512]
                m_mms(pre0[d], d, 0, stop=False)

            for d in (0, 2, 4, 6):
                for i in range(2):
                    pt = ps5.tile([P, 512], F32, tag="p5", name=f"psm{d}_{i}")
                    m_mms(pt, d, i, stop=True)
                    nc.scalar.activation(
                        mt[d][:, i * 512 : (i + 1) * 512], pt[:], Ident,
                        bias=ebt[:, d : d + 1], scale=1.0 / WS,
                    )

            # ---- read: psum[d, t] = WS*(read + m);  out = read + m ----
            def read_psum(i, d, pt, prs, started):
                for idx, j2 in enumerate(prs):
                    nc.tensor.matmul(
                        pt[:],
                        vct[j2][:, :, d * P : (d + 1) * P],
                        et[j2][:, :, i * 512 : (i + 1) * 512],
                        start=(idx == 0 and not started),
                        stop=(idx == len(prs) - 1),
                        perf_mode=DR,
                    )

            def read_evac(i, d, pt, folded):
                osb = osp.tile([P, 512], BF, tag="os", name=f"os{i}_{d}")
                if folded:
                    nc.scalar.activation(
                        osb[:], pt[:], Ident,
                        bias=ebt[:, d : d + 1], scale=1.0 / WS,
                    )
                else:
                    nc.vector.scalar_tensor_tensor(
                        osb[:], pt[:], 1.0 / WS,
                        mt[d][:, i * 512 : (i + 1) * 512], op0=MUL, op1=ADD,
                    )
                nc.sync.dma_start(
                    out_ext[d * P : (d + 1) * P, i * 512 : (i + 1) * 512], osb[:]
                )

            PRS0 = [0, 1, 4, 5]
            PRS1 = [0, 1, 4, 5, 2, 3, 6, 7]
            vc67 = {}
            # read tile 0: odd d pre-folded above
            for d in range(8):
                if d % 2 == 1 and d != 7:
                    pt = pre0[d]
                    read_psum(0, d, pt, PRS0, started=True)
                    read_evac(0, d, pt, folded=True)
                else:
                    pt = ps5.tile([P, 512], F32, tag="p5", name=f"psr0_{d}")
                    if d == 7:
                        m_mms(pt, d, 0, stop=False)
                        read_psum(0, d, pt, PRS0, started=True)
                        read_evac(0, d, pt, folded=True)
                    else:
                        read_psum(0, d, pt, PRS0, started=False)
                        read_evac(0, d, pt, folded=False)
                if d in vc67:
                    vc_half(*vc67[d])
            # read tile 1: folded odd-d psums first (their m-parts keep the
            # PE busy while the late vc pairs drain); last two groups end on
            # different engines (Act for d7, DVE for d6) to shorten the tail
            for d in (1, 3, 5):
                pt = ps5.tile([P, 512], F32, tag="p5", name=f"psr1_{d}")
                m_mms(pt, d, 1, stop=False)
                read_psum(1, d, pt, PRS1, started=True)
                read_evac(1, d, pt, folded=True)
            for d in (0, 2, 4):
                pt = ps1.tile([P, 1024], F32, tag="p1", name=f"psr1_{d}")
                pt = pt[:, 0:512]
                read_psum(1, d, pt, PRS1, started=False)
                read_evac(1, d, pt, folded=False)
            pt = ps5.tile([P, 512], F32, tag="p5", name="psr1_7")
            m_mms(pt, 7, 1, stop=False)
            read_psum(1, 7, pt, PRS1, started=True)
            read_evac(1, 7, pt, folded=True)
            pt = ps1.tile([P, 1024], F32, tag="p1", name="psr1_6")
            pt = pt[:, 0:512]
            read_psum(1, 6, pt, PRS1, started=False)
            read_evac(1, 6, pt, folded=False)

    nc.compile()
    return nc


def _prep_inputs(minibatch, emb_w, emb_b, key_w, key_b, query_w, query_b,
                 value_w, value_b):
    bf = ml_dtypes.bfloat16
    f8 = ml_dtypes.float8_e4m3
    e5 = ml_dtypes.float8_e5m2
    ewT = np.ascontiguousarray(emb_w.T).astype(np.float64)
    W_eq = ewT @ query_w.T.astype(np.float64)
    W_ek = ewT @ key_w.T.astype(np.float64)
    W_ev = (ewT @ value_w.T.astype(np.float64)).astype(np.float32)
    b_ek = emb_b.astype(np.float64) @ key_w.T.astype(np.float64) + key_b.astype(np.float64)
    b_ev = (emb_b @ value_w.T + value_b).astype(np.float32)

    A = W_eq @ W_ek.T               # [HID, HID]; exact in f64
    u = W_eq @ b_ek                 # [HID]; delta[t] = u . x[:, t]

    def pack_w(W):
        # [HID, D] -> [512, 2D]: row 128j+p, col a*D+d  holds W[256j+128a+p, d]
        W4 = np.asarray(W, dtype=np.float32).reshape(4, 2, P, -1)
        return np.ascontiguousarray(
            W4.transpose(0, 2, 1, 3).reshape(512, -1)
        ).astype(f8)

    def pack_bias(v):
        return np.ascontiguousarray(v.reshape(8, P).T).astype(np.float32)

    # 16x scale lifts the ~0.013-rms folded weights out of fp8-subnormal range
    ewT32 = ewT.astype(np.float32)
    mwh_f32 = (WS * ewT32).astype(f8).astype(np.float32)
    shared = {
        "wa8": pack_w(SA * A),
        "wv8": pack_w(WS * W_ev),
        "mwh8": pack_w(mwh_f32),
        "mwl8": pack_w(WS * ewT32 - mwh_f32),
        "vb8": np.concatenate([WS * b_ev, np.zeros(D, np.float32)])[None, :].astype(f8),
        "ones8": np.concatenate(
            [np.ones(P, np.float32), np.zeros(P, np.float32)])[None, :].astype(f8),
        "ident8": np.concatenate(
            [np.eye(P, dtype=np.float32), np.zeros((P, P), np.float32)], axis=1
        ).astype(e5),
    }

    in_maps = []
    for c in range(8):
        b, h = c // 2, c % 2
        xbT = minibatch[b].astype(np.float32)          # [HID, T]
        own = np.concatenate(
            [np.arange(P * (2 * u_ + h), P * (2 * u_ + h) + P) for u_ in range(8)]
        )
        peer = np.concatenate(
            [np.arange(P * (2 * u_ + 1 - h), P * (2 * u_ + 1 - h) + P) for u_ in range(8)]
        )
        xo = xbT[:, own]                               # [HID, 1024] own-t cols
        xpe = xbT[:, peer]

        def pack_x(xc):
            x4 = xc.reshape(4, 2, P, xc.shape[1])
            return np.ascontiguousarray(
                x4.transpose(0, 2, 1, 3).reshape(512, -1)
            ).astype(f8)

        x8o_c = pack_x(xo)
        x8p_c = pack_x(xpe)
        xh = xo.astype(f8).astype(np.float32)
        xl8_c = pack_x(xo - xh)                        # split-fp8 residual of x

        delta = (u @ xo.astype(np.float64)).astype(np.float32)   # [1024]

        # masks: per s-block l (permuted order), boundary-tile content + delta;
        # row SB = delta-only tile for the non-boundary region of bnd=0 blocks;
        # row SB+1 = dummy (matmul'd against the zero half of the identity).
        mk = np.zeros(((SB + 2) * P, 512), dtype=np.float32)
        for l in range(SB):
            bnd = (l % 8) // 4
            base_pos = 4 * bnd
            if l < 8:
                phys = 2 * l + h                 # own-parity s block
            else:
                phys = 2 * (l - 8) + (1 - h)     # peer-parity s block
            srow = P * phys + np.arange(P)[:, None]
            blk = np.zeros((P, 512), dtype=np.float32)
            for pos in range(base_pos, base_pos + 4):
                tcol = P * (2 * pos + h) + np.arange(P)[None, :]
                blk[:, (pos - base_pos) * P : (pos - base_pos + 1) * P] = (
                    (tcol < srow) * NEGM
                )
            blk += delta[bnd * 512 : bnd * 512 + 512][None, :]
            mk[l * P : (l + 1) * P, :] = blk
        mk[SB * P : (SB + 1) * P, :] = delta[512:1024][None, :]

        bias4c = np.zeros((P, 10), dtype=np.float32)
        bias4c[:, 0:8] = pack_bias(emb_b.astype(np.float32))
        bias4c[:, 8] = 1.0 if h == 1 else 0.0    # weight of gathered rank0 rows
        bias4c[:, 9] = 1.0 if h == 0 else 0.0    # weight of gathered rank1 rows
        in_maps.append(dict(
            shared,
            x8o=x8o_c,
            x8p=x8p_c,
            xl8=xl8_c,
            masks=mk.astype(e5),
            bias4=bias4c,
        ))
    return in_maps


def kernel(**inputs):
    global LAST_EXEC_NS
    inputs = {k: np.asarray(v) for k, v in inputs.items()}
    if "nc" not in _CACHE:
        _CACHE["nc"] = _build_nc()
    nc = _CACHE["nc"]
    in_maps = _prep_inputs(**inputs)
    kw = {}
    if PROFILE:
        kw["trace"] = True
    res = run_bass_kernel_spmd(nc, in_maps, core_ids=list(range(8)), **kw)
    LAST_EXEC_NS = getattr(res, "exec_time_ns", None)
    out = np.empty((B, D, T), dtype=np.float32)
    for c in range(8):
        b, h = c // 2, c % 2
        o = np.asarray(res.results[c]["out"]).astype(np.float32)  # [D, OT]
        own = np.concatenate(
            [np.arange(P * (2 * u_ + h), P * (2 * u_ + h) + P) for u_ in range(8)]
        )
        out[b][:, own] = o
    return out
